# revision 18
# baseline (speedup 1.0000x reference)
"""AttentionPairBias Trainium2 kernel (8-core SPMD, row-sharded).

Sharding: core c owns query rows i in [128c, 128c+128) and the matching z
rows. k/v shards are computed from each core's own rows and AllGathered.

z pipeline (phase B): z is shipped pre-transposed from the host as
zT [z, j, i] f16, so no on-device transposes are needed. Per j, one PE
matmul with stationary zT_j [z, i] against wza = [sqrt(128)*w_ln*wz | ones]
yields P' (scaled pair-bias projection) and Sum_z z; a second 1-col matmul
on DVE/ACT-squared z yields Sum_z z^2. LayerNorm then folds in as a
post-matmul correction:
    bias_h(i,j) = rs'_ij * P'_h(i,j) - (m1_ij * rs'_ij) * c1x_h
with rs' = 1/sqrt(var128 + 128*eps), var128 = Sum z^2 - (Sum z)^2/128,
c1x_h = Sum_z wz_dev[z,h]/128. Constant-in-j terms drop (softmax shift
invariance); z_norm_w is folded into wz. No softmax max-subtraction:
logits are O(1) by construction, exact in fp32 exp.
"""
import numpy as np

import concourse.bass as bass
import concourse.tile as tile_mod
from concourse import mybir
from concourse.tile import TileContext
from concourse.masks import make_identity
from concourse.vector_clock import ScopedClock

F32 = mybir.dt.float32
F16 = mybir.dt.float16

S = 1024          # sequence length
DS = 1024         # model dim
H = 16            # heads
HD = 64           # head dim
DZ = 128          # pair dim
NCORES = 8
SI = S // NCORES  # 128 query rows per core

CH = 64           # j's per z DMA chunk
NCH = S // CH     # 16 chunks
SQ = 32           # j's per square block
BK = 16           # j's per P psum bank (16*18 f32 = 1152B < 2KB)
NW = 18           # P bank width: 16 heads + sum(z) + sum(z^2)


# ---------------------------------------------------------------------------
# Framework patch: this walrus build accepts only ONE semaphore wait per
# instruction, but TileContext's final drain aggregates every outstanding sem
# wait onto a single SP Drain. Split the waits across a chain of Drains.
# ---------------------------------------------------------------------------
def _patched_drain_and_barrier(self, tick_clock, wait_clock):
    nc = self.nc
    drain_inst = nc.sync.drain()
    wait_clock.add_sem_waits(
        drain_inst.ins, ScopedClock({None: tick_clock.global_clock})
    )
    si = drain_inst.ins.sync_info
    if si is not None and si.on_wait is not None and len(si.on_wait) > 1:
        extra = list(si.on_wait[1:])
        del si.on_wait[1:]
        for w in extra:
            d2 = nc.sync.drain()
            si2 = d2.ins.sync_info
            if si2 is None:
                d2.ins.sync_info = mybir.SyncInfo(on_wait=[w], on_update=[])
            else:
                si2.on_wait.append(w)
    nc.all_engine_barrier()
    assert self.sems is not None
    popped = nc._tile_sem_poison_stack.pop()
    assert popped is self._sem_poison
    nc.clear_and_free_semaphores(list(self.sems.allocated().values()))
    nc.all_engine_barrier()


def _install_patches():
    tile_mod.TileContext._drain_and_barrier = _patched_drain_and_barrier


_install_patches()


def _split_multiwait(nc):
    """This walrus build accepts at most one semaphore wait per instruction;
    Tile emits more when an op depends on producers on several engines. Hoist
    all-but-one wait onto same-engine NOPs inserted just before. (HW/walrus
    only — CoreSim can't run the unregistered NOPs.)"""
    for fn in nc.m.functions:
        for bb in fn.blocks:
            out = []
            changed = False
            for inst in bb.instructions:
                si = inst.sync_info
                if si is not None and si.on_wait is not None and len(si.on_wait) > 1:
                    extra = list(si.on_wait[:-1])
                    del si.on_wait[:-1]
                    for w in extra:
                        out.append(mybir.InstNoOp(
                            name=nc.get_next_instruction_name(),
                            engine=inst.engine,
                            bass_nofuse=True,
                            sync_info=mybir.SyncInfo(on_wait=[w], on_update=[]),
                        ))
                    changed = True
                out.append(inst)
            if changed:
                bb.instructions[:] = out


def _bcast(ap, dims, extra_offset=0):
    return bass.AP(tensor=ap.tensor, offset=ap.offset + extra_offset, ap=dims)


def build_nc(split_waits=True):
    nc = bass.Bass("TRN2", target_bir_lowering=False, debug=False,
                   num_devices=NCORES)

    zT_sh = nc.dram_tensor("zT_sh", [DZ, S, SI], F16, kind="ExternalInput").ap()
    sTi16 = nc.dram_tensor("sTi16", [DS, SI], F16, kind="ExternalInput").ap()
    wqT16 = nc.dram_tensor("wqT16", [DS, DS], F16, kind="ExternalInput").ap()
    wkT16 = nc.dram_tensor("wkT16", [DS, DS], F16, kind="ExternalInput").ap()
    wvT16 = nc.dram_tensor("wvT16", [DS, DS], F16, kind="ExternalInput").ap()
    wgT16 = nc.dram_tensor("wgT16", [DS, DS], F16, kind="ExternalInput").ap()
    woT16 = nc.dram_tensor("woT16", [DS, DS], F16, kind="ExternalInput").ap()
    wza16 = nc.dram_tensor("wza16", [DZ, NW - 1], F16, kind="ExternalInput").ap()
    c1x = nc.dram_tensor("c1x", [1, H], F32, kind="ExternalInput").ap()
    bq8 = nc.dram_tensor("bq8", [DS, 1], F32, kind="ExternalInput").ap()
    out_sh = nc.dram_tensor("out_sh", [SI, DS], F32, kind="ExternalOutput").ap()

    kv_agi = nc.dram_tensor("kv_agi", [SI, 2 * DS], F16)
    kv_ago = nc.dram_tensor("kv_ago", [S, 2 * DS], F16, addr_space="Shared")

    with TileContext(nc, pool_alloc_mode="queue") as tc:
        _emit(nc, tc, zT_sh, sTi16, wqT16, wkT16, wvT16, wgT16, woT16,
              wza16, c1x, bq8, out_sh, kv_agi, kv_ago)
    if split_waits:
        _split_multiwait(nc)
    return nc


def _emit(nc, tc, zT_sh, sTi16, wqT16, wkT16, wvT16, wgT16, woT16,
          wza16, c1x, bq8, out_sh, kv_agi, kv_ago):
    from contextlib import ExitStack
    AL = mybir.AluOpType
    AF = mybir.ActivationFunctionType

    KT = 8   # 1024/128 K tiles

    ctx = ExitStack()
    with ctx:
        consts = ctx.enter_context(tc.tile_pool(name="consts", bufs=1))
        persist = ctx.enter_context(tc.tile_pool(name="persist", bufs=1))

        ident16 = consts.tile([128, 128], F16)
        make_identity(nc, ident16)
        wza_sb = consts.tile([DZ, NW - 1], F16)   # [z, 16 wz | ones]
        nc.sync.dma_start(out=wza_sb, in_=wza16)
        ones_sb = consts.tile([DZ, 1], F16)
        nc.vector.memset(ones_sb, 1.0)
        c1h_sb = consts.tile([128, H], F32)
        nc.sync.dma_start(out=c1h_sb, in_=_bcast(c1x, [[0, 128], [1, H]]))
        c1m = consts.tile([128, H, BK], F16)      # c1x replicated over j
        nc.gpsimd.tensor_copy(
            c1m, _bcast(c1h_sb, [list(c1h_sb.ap[0]), [1, H], [0, BK]]))
        bq_sb = consts.tile([128, KT], F32)
        nc.sync.dma_start(out=bq_sb, in_=bq8.rearrange("(m p) o -> p (m o)", p=128))
        eps_sb = consts.tile([128, 1], F32)
        nc.vector.memset(eps_sb, 128.0 * 1e-5)

        # persistent SBUF tensors
        kT_sb = persist.tile([128, KT, S], F16)     # [d-part, d-tile, j]
        v_sb = persist.tile([128, KT, DS], F16)     # [j-part, j-tile, d]
        qT_sb = persist.tile([128, KT, SI], F16)    # [d-part, d-tile, i]
        g16 = persist.tile([128, DS], F16)          # [i, d]
        bias16 = persist.tile([128, H, S], F16)     # corrected bias [i, h, j]
        stat = persist.tile([128, S, 2], F32)       # (sum z, sum z^2) per j
        rs = persist.tile([128, S], F32)            # rs' = rs_true/sqrt(128)
        murs = persist.tile([128, S], F16)          # m1 * rs'
        sums = persist.tile([128, H], F32)
        inv = persist.tile([128, H], F32)
        og16 = persist.tile([128, DS], F16)
        ogT_sb = persist.tile([128, KT, SI], F16)
        out_sb = persist.tile([128, DS], F32)

        # ---------------- Phase A: projections + kv AllGather ----------------
        # Weights streamed through 2 rotating 16KB buffers, k/v first so the
        # AllGather can fire early. The first z chunks are DMA'd before phase
        # A's compute so the z pipeline ramps concurrently; the kv staging
        # DMA goes on the gpsimd queue so its wait doesn't block later
        # z-chunk issues on sync.
        apool = ctx.enter_context(tc.tile_pool(name="apool", bufs=1))
        zpool = ctx.enter_context(tc.tile_pool(name="zpool", bufs=3))
        sTi_sb = apool.tile([128, KT, SI], F16)
        nc.sync.dma_start(
            out=sTi_sb, in_=sTi16.rearrange("(m p) n -> p m n", p=128))

        zT_flat = zT_sh.rearrange("z j i -> z (j i)")

        def z_chunk_dma(c):
            zc = zpool.tile([128, CH, DZ], F16, tag="zc", name=f"zc{c}")
            # 2D AP: per-partition the (j, i) range is one contiguous run
            nc.sync.dma_start(
                out=zc.rearrange("p j i -> p (j i)"),
                in_=zT_flat[:, CH * c * SI:CH * (c + 1) * SI])
            return zc

        with (
            tc.tile_pool(name="wpool", bufs=2) as wpool,
            tc.tile_pool(name="apsum", bufs=2, space="PSUM") as apsum,
        ):
            w_sb = {}
            for nm, src in (("wk", wkT16), ("wv", wvT16)):
                w_sb[nm] = wpool.tile([128, KT, DS], F16, tag="w", name=nm)
                nc.sync.dma_start(
                    out=w_sb[nm], in_=src.rearrange("(m p) n -> p m n", p=128))

            zcs = [z_chunk_dma(c) for c in range(3)]

            # k/v shards for own rows: [128 i, 1024 d] each, then AllGather
            kv_sh = apool.tile([128, 2, DS], F16)
            for which, nm in ((0, "wk"), (1, "wv")):
                for n in range(2):
                    kvp = apsum.tile([128, 512], F32, tag="kvp")
                    for k in range(KT):
                        nc.tensor.matmul(kvp, sTi_sb[:, k, :],
                                         w_sb[nm][:, k, 512 * n:512 * (n + 1)],
                                         start=(k == 0), stop=(k == KT - 1))
                    nc.any.tensor_copy(kv_sh[:, which, 512 * n:512 * (n + 1)], kvp)
            nc.gpsimd.dma_start(
                out=kv_agi.ap().rearrange("p (w n) -> p w n", w=2), in_=kv_sh)
            nc.gpsimd.collective_compute(
                "AllGather", AL.bypass, ins=[kv_agi.ap()], outs=[kv_ago.ap()],
                replica_groups=[list(range(NCORES))])

            for nm, src in (("wq", wqT16), ("wg", wgT16)):
                w_sb[nm] = wpool.tile([128, KT, DS], F16, tag="w", name=nm)
                nc.sync.dma_start(
                    out=w_sb[nm], in_=src.rearrange("(m p) n -> p m n", p=128))

            # qT[d, i] += bq  (wq, bq pre-scaled by 1/8 on host)
            for m in range(KT):
                qp = apsum.tile([128, SI], F32, tag="qp")
                for k in range(KT):
                    nc.tensor.matmul(qp, w_sb["wq"][:, k, 128 * m:128 * (m + 1)],
                                     sTi_sb[:, k, :],
                                     start=(k == 0), stop=(k == KT - 1))
                nc.vector.tensor_scalar(
                    out=qT_sb[:, m, :], in0=qp, scalar1=bq_sb[:, m:m + 1],
                    scalar2=None, op0=AL.add)

            # g = sigmoid(s_i @ wg^T)   [i, d]
            for n in range(2):
                gp = apsum.tile([128, 512], F32, tag="gp")
                for k in range(KT):
                    nc.tensor.matmul(gp, sTi_sb[:, k, :],
                                     w_sb["wg"][:, k, 512 * n:512 * (n + 1)],
                                     start=(k == 0), stop=(k == KT - 1))
                nc.scalar.activation(g16[:, 512 * n:512 * (n + 1)], gp,
                                     AF.Sigmoid)

        # ---------------- Phase B: z pipeline ----------------
        with (
            tc.tile_pool(name="sqpool", bufs=2) as sqpool,
            tc.tile_pool(name="ppsum", bufs=6, space="PSUM") as ppsum,
            tc.tile_pool(name="ktps", bufs=2, space="PSUM") as ktps,
            tc.tile_pool(name="stmp", bufs=2) as stmp,
        ):
            def finalize(c):
                # rs' = 1/sqrt(s1 - m1^2/128 + 128 eps); murs = m1 * rs'
                jsl = slice(CH * c, CH * (c + 1))
                m1 = stat[:, jsl, 0:1]
                s1 = stat[:, jsl, 1:2]
                rso = _bcast(rs, [list(rs.ap[0]), [1, CH], [0, 1]],
                             extra_offset=CH * c)
                mo = _bcast(murs, [list(murs.ap[0]), [1, CH], [0, 1]],
                            extra_offset=CH * c)
                t = stmp.tile([128, CH, 1], F32, tag="fin_t")
                nc.vector.tensor_tensor(out=t, in0=m1, in1=m1, op=AL.mult)
                v128 = stmp.tile([128, CH, 1], F32, tag="fin_v")
                nc.vector.scalar_tensor_tensor(
                    out=v128, in0=t, scalar=-1.0 / DZ, in1=s1,
                    op0=AL.mult, op1=AL.add)
                sq = stmp.tile([128, CH, 1], F32, tag="fin_s")
                nc.scalar.activation(sq, v128, AF.Sqrt, bias=eps_sb)
                nc.vector.reciprocal(rso, sq)
                nc.vector.tensor_tensor(out=mo, in0=m1, in1=rso, op=AL.mult)

            def correct(pb, j0):
                # bias16[:, :, j0:j0+BK] = rs*P - murs*c1   (all [128, H, BK])
                rs_rep = _bcast(rs, [list(rs.ap[0]), [0, H], [1, BK]],
                                extra_offset=j0)
                murs_rep = _bcast(murs, [list(murs.ap[0]), [0, H], [1, BK]],
                                  extra_offset=j0)
                pbv = _bcast(pb, [list(pb.ap[0]), [1, H], [NW, BK]])
                t1 = stmp.tile([128, H, BK], F16, tag="t1")
                nc.vector.tensor_tensor(out=t1, in0=pbv, in1=rs_rep, op=AL.mult)
                t2 = stmp.tile([128, H, BK], F16, tag="t2")
                nc.gpsimd.tensor_tensor(out=t2, in0=murs_rep, in1=c1m,
                                        op=AL.mult)
                nc.vector.tensor_tensor(out=bias16[:, :, j0:j0 + BK],
                                        in0=t1, in1=t2, op=AL.subtract)

            for c in range(NCH):
                j0c = CH * c
                zc = zcs[c] if c < 3 else z_chunk_dma(c)

                banks = []
                for s2 in range(CH // SQ):
                    zq = sqpool.tile([128, SQ, DZ], F16, tag="zq")
                    zsl = zc[:, SQ * s2:SQ * (s2 + 1), :]
                    # 12 of 32 blocks on Vector, 20 on Scalar
                    if (2 * c + s2) % 8 in (0, 3, 6):
                        nc.vector.tensor_tensor(out=zq, in0=zsl, in1=zsl,
                                                op=AL.mult)
                    else:
                        nc.scalar.activation(zq, zsl, AF.Square)
                    for b in range(SQ // BK):
                        pb = ppsum.tile([128, BK, NW], F32, tag="pb")
                        jl0 = SQ * s2 + BK * b
                        for jj in range(BK):
                            nc.tensor.matmul(pb[:, jj, 0:NW - 1],
                                             zc[:, jl0 + jj, :], wza_sb,
                                             start=True, stop=True)
                            nc.tensor.matmul(pb[:, jj, NW - 1:NW],
                                             zq[:, jl0 - SQ * s2 + jj, :],
                                             ones_sb, start=True, stop=True)
                        nc.scalar.copy(
                            stat[:, j0c + jl0:j0c + jl0 + BK, :],
                            _bcast(pb, [list(pb.ap[0]), [NW, BK], [1, 2]],
                                   extra_offset=NW - 2))
                        banks.append((pb, j0c + jl0))
                finalize(c)
                for pb, j0 in banks:
                    correct(pb, j0)

                if c == 12:
                    # unpack the gathered k/v; build kT via PE transposes.
                    # Late enough that the AllGather (incl. inter-core skew)
                    # is done — a waiting DMA issue here would block every
                    # later z-chunk issue on the same queue.
                    kv_view = kv_ago.ap().rearrange(
                        "(t p) (w n) -> p t w n", p=128, w=2)
                    nc.sync.dma_start(out=v_sb, in_=kv_view[:, :, 1, :])
                    for m in range(KT):
                        knm = stmp.tile([128, KT, 128], F16, tag="knm")
                        nc.sync.dma_start(
                            out=knm, in_=kv_view[:, :, 0, 128 * m:128 * (m + 1)])
                        ktp = ktps.tile([128, KT, 128], F16, tag="ktp")
                        for t in range(KT):
                            nc.tensor.transpose(ktp[:, t, :], knm[:, t, :],
                                                ident16)
                        nc.any.tensor_copy(
                            kT_sb[:, m, :].rearrange("p (t n) -> p t n", n=128),
                            ktp)

        # ---------------- Phase C: attention ----------------
        with (
            tc.tile_pool(name="scps", bufs=2, space="PSUM") as scps,
            tc.tile_pool(name="atps", bufs=2, space="PSUM") as atps,
            tc.tile_pool(name="ops", bufs=1, space="PSUM") as ops,
            tc.tile_pool(name="attn", bufs=2) as attnp,
        ):
            ob = ops.tile([128, 2, 8, HD], F32)
            for h in range(H):
                m, p0 = h // 2, 64 * (h % 2)
                scp = scps.tile([128, 2, 512], F32, tag="scp")
                for n in range(2):
                    nc.tensor.matmul(scp[:, n, :],
                                     qT_sb[p0:p0 + 64, m, :],
                                     kT_sb[p0:p0 + 64, m, 512 * n:512 * (n + 1)],
                                     start=True, stop=True)
                sc_sb = attnp.tile([128, S], F32, tag="sc")
                nc.vector.tensor_tensor(
                    out=sc_sb, in0=scp.rearrange("p a b -> p (a b)"),
                    in1=bias16[:, h, :], op=AL.add)
                attn16 = attnp.tile([128, S], F16, tag="at")
                nc.scalar.activation(attn16, sc_sb, AF.Exp,
                                     accum_out=sums[:, h:h + 1])
                atb = atps.tile([128, 8, 128], F16, tag="atb")
                for t in range(8):
                    nc.tensor.transpose(atb[:, t, :],
                                        attn16[:, 128 * t:128 * (t + 1)],
                                        ident16)
                attnT = attnp.tile([128, 8, 128], F16, tag="atT")
                nc.any.tensor_copy(attnT, atb)
                for t in range(8):
                    nc.tensor.matmul(ob[:, h // 8, h % 8, :], attnT[:, t, :],
                                     v_sb[:, t, HD * h:HD * (h + 1)],
                                     start=(t == 0), stop=(t == 7))
                if h % 8 == 7:
                    hb = h // 8
                    nc.vector.reciprocal(inv[:, 8 * hb:8 * (hb + 1)],
                                         sums[:, 8 * hb:8 * (hb + 1)])
                    for hh in range(8 * hb, 8 * (hb + 1)):
                        nc.vector.scalar_tensor_tensor(
                            out=og16[:, HD * hh:HD * (hh + 1)],
                            in0=ob[:, hb, hh % 8, :],
                            scalar=inv[:, hh:hh + 1],
                            in1=g16[:, HD * hh:HD * (hh + 1)],
                            op0=AL.mult, op1=AL.mult)

        # ---------------- Phase D: output projection ----------------
        with (
            tc.tile_pool(name="wopool", bufs=1) as wopool,
            tc.tile_pool(name="dpsum", bufs=2, space="PSUM") as dpsum,
        ):
            wo_sb = wopool.tile([128, KT, DS], F16)
            nc.sync.dma_start(
                out=wo_sb, in_=woT16.rearrange("(m p) n -> p m n", p=128))
            ogb = dpsum.tile([128, 8, 128], F16, tag="ogb")
            for t in range(8):
                nc.tensor.transpose(ogb[:, t, :],
                                    og16[:, 128 * t:128 * (t + 1)], ident16)
            nc.any.tensor_copy(ogT_sb.rearrange("p k n -> p (k n)"),
                               ogb.rearrange("p k n -> p (k n)"))
            for n in range(2):
                op_ = dpsum.tile([128, 512], F32, tag="op")
                for k in range(KT):
                    nc.tensor.matmul(op_, ogT_sb[:, k, :],
                                     wo_sb[:, k, 512 * n:512 * (n + 1)],
                                     start=(k == 0), stop=(k == KT - 1))
                nc.any.tensor_copy(out_sb[:, 512 * n:512 * (n + 1)], op_)
            nc.sync.dma_start(out=out_sh, in_=out_sb)


def prep_inputs(s, z, wq, bq, wk, wv, wg, z_norm_w, z_norm_b, wz, wo):
    """Host-side prep: shard + transpose/cast weights and z. Returns in_maps."""
    s2 = np.asarray(s)[0]                     # [S, DS]
    sT = np.ascontiguousarray(s2.T).astype(np.float16)
    wqT = np.ascontiguousarray((np.asarray(wq) / 8.0).T).astype(np.float16)
    wkT = np.ascontiguousarray(np.asarray(wk).T).astype(np.float16)
    wvT = np.ascontiguousarray(np.asarray(wv).T).astype(np.float16)
    wgT = np.ascontiguousarray(np.asarray(wg).T).astype(np.float16)
    woT = np.ascontiguousarray(np.asarray(wo).T).astype(np.float16)
    # wz folded with z_norm_w, scaled by sqrt(DZ); ones column appended.
    wz_f = (np.asarray(z_norm_w)[:, None] * np.asarray(wz).T) * np.sqrt(DZ)
    wz16 = wz_f.astype(np.float16)            # [DZ, H]
    wza = np.concatenate(
        [wz16, np.ones((DZ, 1), np.float16)], axis=1)  # [DZ, H+1]
    # c1x_h = sum_z wz_dev[z, h] / DZ  (f16-quantized wz to match device)
    c1x = (wz16.astype(np.float32).sum(axis=0) / DZ)[None, :].astype(np.float32)
    bq8 = (np.asarray(bq) / 8.0).astype(np.float32)[:, None]

    # z pre-transposed to [z, j, i] per core, f16.
    z0 = np.asarray(z)[0]                     # [S i, S j, DZ z]
    z16 = z0.astype(np.float16)
    zT = z16.transpose(2, 1, 0)               # [DZ z, S j, S i] (view)

    in_maps = []
    for c in range(NCORES):
        i0 = SI * c
        in_maps.append({
            "zT_sh": np.ascontiguousarray(zT[:, :, i0:i0 + SI]),
            "sTi16": np.ascontiguousarray(sT[:, i0:i0 + SI]),
            "wqT16": wqT, "wkT16": wkT, "wvT16": wvT, "wgT16": wgT,
            "woT16": woT, "wza16": wza, "c1x": c1x, "bq8": bq8,
        })
    return in_maps


_NC_CACHE = None


def _get_nc():
    global _NC_CACHE
    if _NC_CACHE is None:
        _NC_CACHE = build_nc()
    return _NC_CACHE


def kernel(**inputs):
    from concourse.bass_utils import run_bass_kernel_spmd
    nc = _get_nc()
    in_maps = prep_inputs(**inputs)
    res = run_bass_kernel_spmd(nc, in_maps, core_ids=list(range(NCORES)))
    out = np.empty((1, S, DS), dtype=np.float32)
    for c in range(NCORES):
        out[0, SI * c:SI * (c + 1), :] = res.results[c]["out_sh"]
    return out


# revision 21
# speedup vs baseline: 1.0585x; 1.0585x over previous
"""AttentionPairBias Trainium2 kernel (8-core SPMD, row-sharded).

Sharding: core c owns query rows i in [128c, 128c+128) and the matching z
rows. k/v shards are computed from each core's own rows and AllGathered.

z pipeline (phase B): z is shipped pre-transposed from the host as
zT [z, j, i] f16, so no on-device transposes are needed. Per j, one PE
matmul with stationary zT_j [z, i] against wza = [sqrt(128)*w_ln*wz | ones]
yields P' (scaled pair-bias projection) and Sum_z z; a second 1-col matmul
on DVE/ACT-squared z yields Sum_z z^2. LayerNorm then folds in as a
post-matmul correction:
    bias_h(i,j) = rs'_ij * P'_h(i,j) - (m1_ij * rs'_ij) * c1x_h
with rs' = 1/sqrt(var128 + 128*eps), var128 = Sum z^2 - (Sum z)^2/128,
c1x_h = Sum_z wz_dev[z,h]/128. Constant-in-j terms drop (softmax shift
invariance); z_norm_w is folded into wz. No softmax max-subtraction:
logits are O(1) by construction, exact in fp32 exp.
"""
import numpy as np

import concourse.bass as bass
import concourse.tile as tile_mod
from concourse import mybir
from concourse.tile import TileContext
from concourse.masks import make_identity
from concourse.vector_clock import ScopedClock

F32 = mybir.dt.float32
F16 = mybir.dt.float16

S = 1024          # sequence length
DS = 1024         # model dim
H = 16            # heads
HD = 64           # head dim
DZ = 128          # pair dim
NCORES = 8
SI = S // NCORES  # 128 query rows per core

CH = 64           # j's per z DMA chunk
NCH = S // CH     # 16 chunks
SQ = 32           # j's per square block
BK = 16           # j's per P psum bank (16*18 f32 = 1152B < 2KB)
NW = 18           # P bank width: 16 heads + sum(z) + sum(z^2)


# ---------------------------------------------------------------------------
# Framework patch: this walrus build accepts only ONE semaphore wait per
# instruction, but TileContext's final drain aggregates every outstanding sem
# wait onto a single SP Drain. Split the waits across a chain of Drains.
# ---------------------------------------------------------------------------
def _patched_drain_and_barrier(self, tick_clock, wait_clock):
    nc = self.nc
    drain_inst = nc.sync.drain()
    wait_clock.add_sem_waits(
        drain_inst.ins, ScopedClock({None: tick_clock.global_clock})
    )
    si = drain_inst.ins.sync_info
    if si is not None and si.on_wait is not None and len(si.on_wait) > 1:
        extra = list(si.on_wait[1:])
        del si.on_wait[1:]
        for w in extra:
            d2 = nc.sync.drain()
            si2 = d2.ins.sync_info
            if si2 is None:
                d2.ins.sync_info = mybir.SyncInfo(on_wait=[w], on_update=[])
            else:
                si2.on_wait.append(w)
    nc.all_engine_barrier()
    assert self.sems is not None
    popped = nc._tile_sem_poison_stack.pop()
    assert popped is self._sem_poison
    nc.clear_and_free_semaphores(list(self.sems.allocated().values()))
    nc.all_engine_barrier()


def _install_patches():
    tile_mod.TileContext._drain_and_barrier = _patched_drain_and_barrier


_install_patches()


def _split_multiwait(nc):
    """This walrus build accepts at most one semaphore wait per instruction;
    Tile emits more when an op depends on producers on several engines. Hoist
    all-but-one wait onto same-engine NOPs inserted just before. (HW/walrus
    only — CoreSim can't run the unregistered NOPs.)"""
    for fn in nc.m.functions:
        for bb in fn.blocks:
            out = []
            changed = False
            for inst in bb.instructions:
                si = inst.sync_info
                if si is not None and si.on_wait is not None and len(si.on_wait) > 1:
                    extra = list(si.on_wait[:-1])
                    del si.on_wait[:-1]
                    for w in extra:
                        out.append(mybir.InstNoOp(
                            name=nc.get_next_instruction_name(),
                            engine=inst.engine,
                            bass_nofuse=True,
                            sync_info=mybir.SyncInfo(on_wait=[w], on_update=[]),
                        ))
                    changed = True
                out.append(inst)
            if changed:
                bb.instructions[:] = out


def _bcast(ap, dims, extra_offset=0):
    return bass.AP(tensor=ap.tensor, offset=ap.offset + extra_offset, ap=dims)


def build_nc(split_waits=True):
    nc = bass.Bass("TRN2", target_bir_lowering=False, debug=False,
                   num_devices=NCORES)

    zT_sh = nc.dram_tensor("zT_sh", [DZ, S, SI], F16, kind="ExternalInput").ap()
    sTi16 = nc.dram_tensor("sTi16", [DS, SI], F16, kind="ExternalInput").ap()
    wqT16 = nc.dram_tensor("wqT16", [DS, DS], F16, kind="ExternalInput").ap()
    wkT16 = nc.dram_tensor("wkT16", [DS, DS], F16, kind="ExternalInput").ap()
    wvT16 = nc.dram_tensor("wvT16", [DS, DS], F16, kind="ExternalInput").ap()
    wgT16 = nc.dram_tensor("wgT16", [DS, DS], F16, kind="ExternalInput").ap()
    woT16 = nc.dram_tensor("woT16", [DS, DS], F16, kind="ExternalInput").ap()
    wza16 = nc.dram_tensor("wza16", [DZ, NW - 1], F16, kind="ExternalInput").ap()
    c1x = nc.dram_tensor("c1x", [1, H], F32, kind="ExternalInput").ap()
    bq8 = nc.dram_tensor("bq8", [DS, 1], F32, kind="ExternalInput").ap()
    out_sh = nc.dram_tensor("out_sh", [SI, DS], F32, kind="ExternalOutput").ap()

    kv_agi = nc.dram_tensor("kv_agi", [SI, 2 * DS], F16)
    kv_ago = nc.dram_tensor("kv_ago", [S, 2 * DS], F16, addr_space="Shared")

    with TileContext(nc, pool_alloc_mode="queue") as tc:
        _emit(nc, tc, zT_sh, sTi16, wqT16, wkT16, wvT16, wgT16, woT16,
              wza16, c1x, bq8, out_sh, kv_agi, kv_ago)
    if split_waits:
        _split_multiwait(nc)
    return nc


def _emit(nc, tc, zT_sh, sTi16, wqT16, wkT16, wvT16, wgT16, woT16,
          wza16, c1x, bq8, out_sh, kv_agi, kv_ago):
    from contextlib import ExitStack
    AL = mybir.AluOpType
    AF = mybir.ActivationFunctionType

    KT = 8   # 1024/128 K tiles

    ctx = ExitStack()
    with ctx:
        consts = ctx.enter_context(tc.tile_pool(name="consts", bufs=1))
        persist = ctx.enter_context(tc.tile_pool(name="persist", bufs=1))

        ident16 = consts.tile([128, 128], F16)
        make_identity(nc, ident16)
        wza_sb = consts.tile([DZ, NW - 1], F16)   # [z, 16 wz | ones]
        nc.sync.dma_start(out=wza_sb, in_=wza16)
        ones_sb = consts.tile([DZ, 1], F16)
        nc.vector.memset(ones_sb, 1.0)
        c1h_sb = consts.tile([128, H], F32)
        nc.sync.dma_start(out=c1h_sb, in_=_bcast(c1x, [[0, 128], [1, H]]))
        c1m = consts.tile([128, H, BK], F16)      # c1x replicated over j
        nc.gpsimd.tensor_copy(
            c1m, _bcast(c1h_sb, [list(c1h_sb.ap[0]), [1, H], [0, BK]]))
        bq_sb = consts.tile([128, KT], F32)
        nc.sync.dma_start(out=bq_sb, in_=bq8.rearrange("(m p) o -> p (m o)", p=128))
        eps_sb = consts.tile([128, 1], F32)
        nc.vector.memset(eps_sb, 128.0 * 1e-5)

        # persistent SBUF tensors
        kT_sb = persist.tile([128, KT, S], F16)     # [d-part, d-tile, j]
        v_sb = persist.tile([128, KT, DS], F16)     # [j-part, j-tile, d]
        qT_sb = persist.tile([128, KT, SI], F16)    # [d-part, d-tile, i]
        g16 = persist.tile([128, DS], F16)          # [i, d]
        bias16 = persist.tile([128, H, S], F16)     # corrected bias [i, h, j]
        stat = persist.tile([128, S, 2], F32)       # (sum z, sum z^2) per j
        rs = persist.tile([128, S], F32)            # rs' = rs_true/sqrt(128)
        murs = persist.tile([128, S], F16)          # m1 * rs'
        sums = persist.tile([128, H], F32)
        inv = persist.tile([128, H], F32)
        og16 = persist.tile([128, DS], F16)
        ogT_sb = persist.tile([128, KT, SI], F16)
        out_sb = persist.tile([128, DS], F32)

        # ---------------- Phase A: projections + kv AllGather ----------------
        # Weights streamed through 2 rotating 16KB buffers, k/v first so the
        # AllGather can fire early. The first z chunks are DMA'd before phase
        # A's compute so the z pipeline ramps concurrently; the kv staging
        # DMA goes on the gpsimd queue so its wait doesn't block later
        # z-chunk issues on sync.
        apool = ctx.enter_context(tc.tile_pool(name="apool", bufs=1))
        zpool = ctx.enter_context(tc.tile_pool(name="zpool", bufs=4))
        sTi_sb = apool.tile([128, KT, SI], F16)
        nc.sync.dma_start(
            out=sTi_sb, in_=sTi16.rearrange("(m p) n -> p m n", p=128))

        zT_flat = zT_sh.rearrange("z j i -> z (j i)")

        def z_chunk_dma(c):
            zc = zpool.tile([128, CH, DZ], F16, tag="zc", name=f"zc{c}")
            # 2D AP: per-partition the (j, i) range is one contiguous run
            nc.sync.dma_start(
                out=zc.rearrange("p j i -> p (j i)"),
                in_=zT_flat[:, CH * c * SI:CH * (c + 1) * SI])
            return zc

        with (
            tc.tile_pool(name="wpool", bufs=2) as wpool,
            tc.tile_pool(name="apsum", bufs=2, space="PSUM") as apsum,
        ):
            w_sb = {}
            for nm, src in (("wk", wkT16), ("wv", wvT16)):
                w_sb[nm] = wpool.tile([128, KT, DS], F16, tag="w", name=nm)
                nc.sync.dma_start(
                    out=w_sb[nm], in_=src.rearrange("(m p) n -> p m n", p=128))

            zcs = [z_chunk_dma(c) for c in range(4)]

            # k/v shards for own rows: [128 i, 1024 d] each, then AllGather
            kv_sh = apool.tile([128, 2, DS], F16)
            for which, nm in ((0, "wk"), (1, "wv")):
                for n in range(2):
                    kvp = apsum.tile([128, 512], F32, tag="kvp")
                    for k in range(KT):
                        nc.tensor.matmul(kvp, sTi_sb[:, k, :],
                                         w_sb[nm][:, k, 512 * n:512 * (n + 1)],
                                         start=(k == 0), stop=(k == KT - 1))
                    nc.any.tensor_copy(kv_sh[:, which, 512 * n:512 * (n + 1)], kvp)

            for nm, src in (("wq", wqT16), ("wg", wgT16)):
                w_sb[nm] = wpool.tile([128, KT, DS], F16, tag="w", name=nm)
                nc.sync.dma_start(
                    out=w_sb[nm], in_=src.rearrange("(m p) n -> p m n", p=128))

            nc.sync.dma_start(
                out=kv_agi.ap().rearrange("p (w n) -> p w n", w=2), in_=kv_sh)
            nc.gpsimd.collective_compute(
                "AllGather", AL.bypass, ins=[kv_agi.ap()], outs=[kv_ago.ap()],
                replica_groups=[list(range(NCORES))])

            # qT[d, i] += bq  (wq, bq pre-scaled by 1/8 on host)
            for m in range(KT):
                qp = apsum.tile([128, SI], F32, tag="qp")
                for k in range(KT):
                    nc.tensor.matmul(qp, w_sb["wq"][:, k, 128 * m:128 * (m + 1)],
                                     sTi_sb[:, k, :],
                                     start=(k == 0), stop=(k == KT - 1))
                nc.vector.tensor_scalar(
                    out=qT_sb[:, m, :], in0=qp, scalar1=bq_sb[:, m:m + 1],
                    scalar2=None, op0=AL.add)

            # g = sigmoid(s_i @ wg^T)   [i, d]
            for n in range(2):
                gp = apsum.tile([128, 512], F32, tag="gp")
                for k in range(KT):
                    nc.tensor.matmul(gp, sTi_sb[:, k, :],
                                     w_sb["wg"][:, k, 512 * n:512 * (n + 1)],
                                     start=(k == 0), stop=(k == KT - 1))
                nc.scalar.activation(g16[:, 512 * n:512 * (n + 1)], gp,
                                     AF.Sigmoid)

        # ---------------- Phase B: z pipeline ----------------
        with (
            tc.tile_pool(name="sqpool", bufs=2) as sqpool,
            tc.tile_pool(name="ppsum", bufs=6, space="PSUM") as ppsum,
            tc.tile_pool(name="ktps", bufs=2, space="PSUM") as ktps,
            tc.tile_pool(name="stmp", bufs=2) as stmp,
        ):
            def finalize(c):
                # rs' = 1/sqrt(s1 - m1^2/128 + 128 eps); murs = m1 * rs'
                jsl = slice(CH * c, CH * (c + 1))
                m1 = stat[:, jsl, 0:1]
                s1 = stat[:, jsl, 1:2]
                rso = _bcast(rs, [list(rs.ap[0]), [1, CH], [0, 1]],
                             extra_offset=CH * c)
                mo = _bcast(murs, [list(murs.ap[0]), [1, CH], [0, 1]],
                            extra_offset=CH * c)
                t = stmp.tile([128, CH, 1], F32, tag="fin_t")
                nc.vector.tensor_tensor(out=t, in0=m1, in1=m1, op=AL.mult)
                v128 = stmp.tile([128, CH, 1], F32, tag="fin_v")
                nc.vector.scalar_tensor_tensor(
                    out=v128, in0=t, scalar=-1.0 / DZ, in1=s1,
                    op0=AL.mult, op1=AL.add)
                sq = stmp.tile([128, CH, 1], F32, tag="fin_s")
                nc.scalar.activation(sq, v128, AF.Sqrt, bias=eps_sb)
                nc.vector.reciprocal(rso, sq)
                nc.vector.tensor_tensor(out=mo, in0=m1, in1=rso, op=AL.mult)

            def correct(pb, j0):
                # bias16[:, :, j0:j0+BK] = rs*P - murs*c1   (all [128, H, BK])
                rs_rep = _bcast(rs, [list(rs.ap[0]), [0, H], [1, BK]],
                                extra_offset=j0)
                murs_rep = _bcast(murs, [list(murs.ap[0]), [0, H], [1, BK]],
                                  extra_offset=j0)
                pbv = _bcast(pb, [list(pb.ap[0]), [1, H], [NW, BK]])
                t1 = stmp.tile([128, H, BK], F16, tag="t1")
                nc.vector.tensor_tensor(out=t1, in0=pbv, in1=rs_rep, op=AL.mult)
                t2 = stmp.tile([128, H, BK], F16, tag="t2")
                nc.gpsimd.tensor_tensor(out=t2, in0=murs_rep, in1=c1m,
                                        op=AL.mult)
                nc.vector.tensor_tensor(out=bias16[:, :, j0:j0 + BK],
                                        in0=t1, in1=t2, op=AL.subtract)

            for c in range(NCH):
                j0c = CH * c
                zc = zcs[c] if c < 4 else z_chunk_dma(c)

                banks = []
                for s2 in range(CH // SQ):
                    zq = sqpool.tile([128, SQ, DZ], F16, tag="zq")
                    zsl = zc[:, SQ * s2:SQ * (s2 + 1), :]
                    # 12 of 32 blocks on Vector, 20 on Scalar
                    if (2 * c + s2) % 8 in (0, 3, 6):
                        nc.vector.tensor_tensor(out=zq, in0=zsl, in1=zsl,
                                                op=AL.mult)
                    else:
                        nc.scalar.activation(zq, zsl, AF.Square)
                    for b in range(SQ // BK):
                        pb = ppsum.tile([128, BK, NW], F32, tag="pb")
                        jl0 = SQ * s2 + BK * b
                        for jj in range(BK):
                            nc.tensor.matmul(pb[:, jj, 0:NW - 1],
                                             zc[:, jl0 + jj, :], wza_sb,
                                             start=True, stop=True)
                            nc.tensor.matmul(pb[:, jj, NW - 1:NW],
                                             zq[:, jl0 - SQ * s2 + jj, :],
                                             ones_sb, start=True, stop=True)
                        nc.scalar.copy(
                            stat[:, j0c + jl0:j0c + jl0 + BK, :],
                            _bcast(pb, [list(pb.ap[0]), [NW, BK], [1, 2]],
                                   extra_offset=NW - 2))
                        banks.append((pb, j0c + jl0))
                finalize(c)
                for pb, j0 in banks:
                    correct(pb, j0)

                if c == 12:
                    # unpack the gathered k/v; build kT via PE transposes.
                    # Late enough that the AllGather (incl. inter-core skew)
                    # is done — a waiting DMA issue here would block every
                    # later z-chunk issue on the same queue.
                    kv_view = kv_ago.ap().rearrange(
                        "(t p) (w n) -> p t w n", p=128, w=2)
                    nc.sync.dma_start(out=v_sb, in_=kv_view[:, :, 1, :])
                    for m in range(KT):
                        knm = stmp.tile([128, KT, 128], F16, tag="knm")
                        nc.sync.dma_start(
                            out=knm, in_=kv_view[:, :, 0, 128 * m:128 * (m + 1)])
                        ktp = ktps.tile([128, KT, 128], F16, tag="ktp")
                        for t in range(KT):
                            nc.tensor.transpose(ktp[:, t, :], knm[:, t, :],
                                                ident16)
                        nc.any.tensor_copy(
                            kT_sb[:, m, :].rearrange("p (t n) -> p t n", n=128),
                            ktp)

        # ---------------- Phase C: attention ----------------
        with (
            tc.tile_pool(name="scps", bufs=2, space="PSUM") as scps,
            tc.tile_pool(name="atps", bufs=2, space="PSUM") as atps,
            tc.tile_pool(name="ops", bufs=1, space="PSUM") as ops,
            tc.tile_pool(name="attn", bufs=2) as attnp,
        ):
            ob = ops.tile([128, 2, 8, HD], F32)
            for h in range(H):
                m, p0 = h // 2, 64 * (h % 2)
                scp = scps.tile([128, 2, 512], F32, tag="scp")
                for n in range(2):
                    nc.tensor.matmul(scp[:, n, :],
                                     qT_sb[p0:p0 + 64, m, :],
                                     kT_sb[p0:p0 + 64, m, 512 * n:512 * (n + 1)],
                                     start=True, stop=True)
                sc_sb = attnp.tile([128, S], F32, tag="sc")
                nc.vector.tensor_tensor(
                    out=sc_sb, in0=scp.rearrange("p a b -> p (a b)"),
                    in1=bias16[:, h, :], op=AL.add)
                attn16 = attnp.tile([128, S], F16, tag="at")
                nc.scalar.activation(attn16, sc_sb, AF.Exp,
                                     accum_out=sums[:, h:h + 1])
                atb = atps.tile([128, 8, 128], F16, tag="atb")
                for t in range(8):
                    nc.tensor.transpose(atb[:, t, :],
                                        attn16[:, 128 * t:128 * (t + 1)],
                                        ident16)
                attnT = attnp.tile([128, 8, 128], F16, tag="atT")
                nc.any.tensor_copy(attnT, atb)
                for t in range(8):
                    nc.tensor.matmul(ob[:, h // 8, h % 8, :], attnT[:, t, :],
                                     v_sb[:, t, HD * h:HD * (h + 1)],
                                     start=(t == 0), stop=(t == 7))
                if h % 8 == 7:
                    hb = h // 8
                    nc.vector.reciprocal(inv[:, 8 * hb:8 * (hb + 1)],
                                         sums[:, 8 * hb:8 * (hb + 1)])
                    for hh in range(8 * hb, 8 * (hb + 1)):
                        nc.vector.scalar_tensor_tensor(
                            out=og16[:, HD * hh:HD * (hh + 1)],
                            in0=ob[:, hb, hh % 8, :],
                            scalar=inv[:, hh:hh + 1],
                            in1=g16[:, HD * hh:HD * (hh + 1)],
                            op0=AL.mult, op1=AL.mult)

        # ---------------- Phase D: output projection ----------------
        with (
            tc.tile_pool(name="wopool", bufs=1) as wopool,
            tc.tile_pool(name="dpsum", bufs=2, space="PSUM") as dpsum,
        ):
            wo_sb = wopool.tile([128, KT, DS], F16)
            nc.sync.dma_start(
                out=wo_sb, in_=woT16.rearrange("(m p) n -> p m n", p=128))
            ogb = dpsum.tile([128, 8, 128], F16, tag="ogb")
            for t in range(8):
                nc.tensor.transpose(ogb[:, t, :],
                                    og16[:, 128 * t:128 * (t + 1)], ident16)
            nc.any.tensor_copy(ogT_sb.rearrange("p k n -> p (k n)"),
                               ogb.rearrange("p k n -> p (k n)"))
            for n in range(2):
                op_ = dpsum.tile([128, 512], F32, tag="op")
                for k in range(KT):
                    nc.tensor.matmul(op_, ogT_sb[:, k, :],
                                     wo_sb[:, k, 512 * n:512 * (n + 1)],
                                     start=(k == 0), stop=(k == KT - 1))
                nc.any.tensor_copy(out_sb[:, 512 * n:512 * (n + 1)], op_)
            nc.sync.dma_start(out=out_sh, in_=out_sb)


def prep_inputs(s, z, wq, bq, wk, wv, wg, z_norm_w, z_norm_b, wz, wo):
    """Host-side prep: shard + transpose/cast weights and z. Returns in_maps."""
    s2 = np.asarray(s)[0]                     # [S, DS]
    sT = np.ascontiguousarray(s2.T).astype(np.float16)
    wqT = np.ascontiguousarray((np.asarray(wq) / 8.0).T).astype(np.float16)
    wkT = np.ascontiguousarray(np.asarray(wk).T).astype(np.float16)
    wvT = np.ascontiguousarray(np.asarray(wv).T).astype(np.float16)
    wgT = np.ascontiguousarray(np.asarray(wg).T).astype(np.float16)
    woT = np.ascontiguousarray(np.asarray(wo).T).astype(np.float16)
    # wz folded with z_norm_w, scaled by sqrt(DZ); ones column appended.
    wz_f = (np.asarray(z_norm_w)[:, None] * np.asarray(wz).T) * np.sqrt(DZ)
    wz16 = wz_f.astype(np.float16)            # [DZ, H]
    wza = np.concatenate(
        [wz16, np.ones((DZ, 1), np.float16)], axis=1)  # [DZ, H+1]
    # c1x_h = sum_z wz_dev[z, h] / DZ  (f16-quantized wz to match device)
    c1x = (wz16.astype(np.float32).sum(axis=0) / DZ)[None, :].astype(np.float32)
    bq8 = (np.asarray(bq) / 8.0).astype(np.float32)[:, None]

    # z pre-transposed to [z, j, i] per core, f16.
    z0 = np.asarray(z)[0]                     # [S i, S j, DZ z]
    z16 = z0.astype(np.float16)
    zT = z16.transpose(2, 1, 0)               # [DZ z, S j, S i] (view)

    in_maps = []
    for c in range(NCORES):
        i0 = SI * c
        in_maps.append({
            "zT_sh": np.ascontiguousarray(zT[:, :, i0:i0 + SI]),
            "sTi16": np.ascontiguousarray(sT[:, i0:i0 + SI]),
            "wqT16": wqT, "wkT16": wkT, "wvT16": wvT, "wgT16": wgT,
            "woT16": woT, "wza16": wza, "c1x": c1x, "bq8": bq8,
        })
    return in_maps


_NC_CACHE = None


def _get_nc():
    global _NC_CACHE
    if _NC_CACHE is None:
        _NC_CACHE = build_nc()
    return _NC_CACHE


def kernel(**inputs):
    from concourse.bass_utils import run_bass_kernel_spmd
    nc = _get_nc()
    in_maps = prep_inputs(**inputs)
    res = run_bass_kernel_spmd(nc, in_maps, core_ids=list(range(NCORES)))
    out = np.empty((1, S, DS), dtype=np.float32)
    for c in range(NCORES):
        out[0, SI * c:SI * (c + 1), :] = res.results[c]["out_sh"]
    return out


# revision 34
# speedup vs baseline: 1.0711x; 1.0119x over previous
"""AttentionPairBias Trainium2 kernel (8-core SPMD, row-sharded).

Sharding: core c owns query rows i in [128c, 128c+128) and the matching z
rows. k/v shards are computed from each core's own rows and AllGathered.

z pipeline (phase B): z is shipped pre-transposed from the host as
zT [z, j, i] f16, so no on-device transposes are needed. Per j, one PE
matmul with stationary zT_j [z, i] against wza = [sqrt(128)*w_ln*wz | ones]
yields P' (scaled pair-bias projection) and Sum_z z; a second 1-col matmul
on DVE/ACT-squared z yields Sum_z z^2. LayerNorm then folds in as a
post-matmul correction:
    bias_h(i,j) = rs'_ij * P'_h(i,j) - (m1_ij * rs'_ij) * c1x_h
with rs' = 1/sqrt(var128 + 128*eps), var128 = Sum z^2 - (Sum z)^2/128,
c1x_h = Sum_z wz_dev[z,h]/128. Constant-in-j terms drop (softmax shift
invariance); z_norm_w is folded into wz. No softmax max-subtraction:
logits are O(1) by construction, exact in fp32 exp.
"""
import numpy as np

import concourse.bass as bass
import concourse.tile as tile_mod
from concourse import mybir
from concourse.tile import TileContext
from concourse.masks import make_identity
from concourse.vector_clock import ScopedClock

F32 = mybir.dt.float32
F16 = mybir.dt.float16

S = 1024          # sequence length
DS = 1024         # model dim
H = 16            # heads
HD = 64           # head dim
DZ = 128          # pair dim
NCORES = 8
SI = S // NCORES  # 128 query rows per core

KT0 = 8           # 1024/128 K tiles (module-level; _emit uses KT)
CH = 64           # j's per z DMA chunk
NCH = S // CH     # 16 chunks
SQ = 32           # j's per square block
BK = 16           # j's per P psum bank (16*18 f32 = 1152B < 2KB)
NW = 18           # P bank width: 16 heads + sum(z) + sum(z^2)


# ---------------------------------------------------------------------------
# Framework patch: this walrus build accepts only ONE semaphore wait per
# instruction, but TileContext's final drain aggregates every outstanding sem
# wait onto a single SP Drain. Split the waits across a chain of Drains.
# ---------------------------------------------------------------------------
def _patched_drain_and_barrier(self, tick_clock, wait_clock):
    nc = self.nc
    drain_inst = nc.sync.drain()
    wait_clock.add_sem_waits(
        drain_inst.ins, ScopedClock({None: tick_clock.global_clock})
    )
    si = drain_inst.ins.sync_info
    if si is not None and si.on_wait is not None and len(si.on_wait) > 1:
        extra = list(si.on_wait[1:])
        del si.on_wait[1:]
        for w in extra:
            d2 = nc.sync.drain()
            si2 = d2.ins.sync_info
            if si2 is None:
                d2.ins.sync_info = mybir.SyncInfo(on_wait=[w], on_update=[])
            else:
                si2.on_wait.append(w)
    nc.all_engine_barrier()
    assert self.sems is not None
    popped = nc._tile_sem_poison_stack.pop()
    assert popped is self._sem_poison
    nc.clear_and_free_semaphores(list(self.sems.allocated().values()))
    nc.all_engine_barrier()


def _install_patches():
    tile_mod.TileContext._drain_and_barrier = _patched_drain_and_barrier


_install_patches()


def _split_multiwait(nc):
    """This walrus build accepts at most one semaphore wait per instruction;
    Tile emits more when an op depends on producers on several engines. Hoist
    all-but-one wait onto same-engine NOPs inserted just before. (HW/walrus
    only — CoreSim can't run the unregistered NOPs.)"""
    for fn in nc.m.functions:
        for bb in fn.blocks:
            out = []
            changed = False
            for inst in bb.instructions:
                si = inst.sync_info
                if si is not None and si.on_wait is not None and len(si.on_wait) > 1:
                    extra = list(si.on_wait[:-1])
                    del si.on_wait[:-1]
                    for w in extra:
                        out.append(mybir.InstNoOp(
                            name=nc.get_next_instruction_name(),
                            engine=inst.engine,
                            bass_nofuse=True,
                            sync_info=mybir.SyncInfo(on_wait=[w], on_update=[]),
                        ))
                    changed = True
                out.append(inst)
            if changed:
                bb.instructions[:] = out


def _bcast(ap, dims, extra_offset=0):
    return bass.AP(tensor=ap.tensor, offset=ap.offset + extra_offset, ap=dims)


def build_nc(split_waits=True):
    nc = bass.Bass("TRN2", target_bir_lowering=False, debug=False,
                   num_devices=NCORES)

    # weights are host-packed to [128, KT*DS] so each loads with one
    # 128-descriptor contiguous DMA (16KB per partition)
    zT_sh = nc.dram_tensor("zT_sh", [DZ, S, SI], F16, kind="ExternalInput").ap()
    sTi16 = nc.dram_tensor("sTi16", [128, KT0 * SI], F16,
                           kind="ExternalInput").ap()
    wqT16 = nc.dram_tensor("wqT16", [128, KT0 * DS], F16,
                           kind="ExternalInput").ap()
    wkT16 = nc.dram_tensor("wkT16", [128, KT0 * DS], F16,
                           kind="ExternalInput").ap()
    wvT16 = nc.dram_tensor("wvT16", [128, KT0 * DS], F16,
                           kind="ExternalInput").ap()
    wgT16 = nc.dram_tensor("wgT16", [128, KT0 * DS], F16,
                           kind="ExternalInput").ap()
    woT16 = nc.dram_tensor("woT16", [128, KT0 * DS], F16,
                           kind="ExternalInput").ap()
    wza16 = nc.dram_tensor("wza16", [DZ, NW - 1], F16, kind="ExternalInput").ap()
    c1x = nc.dram_tensor("c1x", [1, H], F32, kind="ExternalInput").ap()
    bq8 = nc.dram_tensor("bq8", [DS, 1], F32, kind="ExternalInput").ap()
    out_sh = nc.dram_tensor("out_sh", [SI, DS], F32, kind="ExternalOutput").ap()

    kv_agi = nc.dram_tensor("kv_agi", [SI, 2 * DS], F16)
    kv_ago = nc.dram_tensor("kv_ago", [S, 2 * DS], F16, addr_space="Shared")

    with TileContext(nc, pool_alloc_mode="queue") as tc:
        _emit(nc, tc, zT_sh, sTi16, wqT16, wkT16, wvT16, wgT16, woT16,
              wza16, c1x, bq8, out_sh, kv_agi, kv_ago)
    if split_waits:
        _split_multiwait(nc)
    return nc


def _emit(nc, tc, zT_sh, sTi16, wqT16, wkT16, wvT16, wgT16, woT16,
          wza16, c1x, bq8, out_sh, kv_agi, kv_ago):
    from contextlib import ExitStack
    AL = mybir.AluOpType
    AF = mybir.ActivationFunctionType

    KT = 8   # 1024/128 K tiles

    ctx = ExitStack()
    with ctx:
        consts = ctx.enter_context(tc.tile_pool(name="consts", bufs=1))
        persist = ctx.enter_context(tc.tile_pool(name="persist", bufs=1))

        ident16 = consts.tile([128, 128], F16)
        make_identity(nc, ident16)
        wza_sb = consts.tile([DZ, NW - 1], F16)   # [z, 16 wz | ones]
        nc.sync.dma_start(out=wza_sb, in_=wza16)
        ones_sb = consts.tile([DZ, 1], F16)
        nc.vector.memset(ones_sb, 1.0)
        c1h_sb = consts.tile([128, H], F32)
        nc.sync.dma_start(out=c1h_sb, in_=_bcast(c1x, [[0, 128], [1, H]]))
        c1m = consts.tile([128, H, BK], F16)      # c1x replicated over j
        nc.gpsimd.tensor_copy(
            c1m, _bcast(c1h_sb, [list(c1h_sb.ap[0]), [1, H], [0, BK]]))
        bq_sb = consts.tile([128, KT], F32)
        nc.sync.dma_start(out=bq_sb, in_=bq8.rearrange("(m p) o -> p (m o)", p=128))
        eps_sb = consts.tile([128, 1], F32)
        nc.vector.memset(eps_sb, 128.0 * 1e-5)

        # persistent SBUF tensors
        kT_sb = persist.tile([128, KT, S], F16)     # [d-part, d-tile, j]
        v_sb = persist.tile([128, KT, DS], F16)     # [j-part, j-tile, d]
        qT_sb = persist.tile([128, KT, SI], F16)    # [d-part, d-tile, i]
        g16 = persist.tile([128, DS], F16)          # [i, d]
        bias16 = persist.tile([128, H, S], F16)     # corrected bias [i, h, j]
        stat = persist.tile([128, S, 2], F32)       # (sum z, sum z^2) per j
        rs = persist.tile([128, S], F32)            # rs' = rs_true/sqrt(128)
        murs = persist.tile([128, S], F16)          # m1 * rs'
        sums = persist.tile([128, H], F32)
        inv = persist.tile([128, H], F32)
        og16 = persist.tile([128, DS], F16)
        ogT_sb = persist.tile([128, KT, SI], F16)
        out_sb = persist.tile([128, DS], F32)

        # ---------------- Phase A: projections + kv AllGather ----------------
        # Weights streamed through 2 rotating 16KB buffers, k/v first so the
        # AllGather can fire early. The first z chunks are DMA'd before phase
        # A's compute so the z pipeline ramps concurrently; the kv staging
        # DMA goes on the gpsimd queue so its wait doesn't block later
        # z-chunk issues on sync.
        apool = ctx.enter_context(tc.tile_pool(name="apool", bufs=1))
        zpool = ctx.enter_context(tc.tile_pool(name="zpool", bufs=4))
        sTi_sb = apool.tile([128, KT, SI], F16)
        nc.sync.dma_start(
            out=sTi_sb.rearrange("p m n -> p (m n)"), in_=sTi16)

        zT_flat = zT_sh.rearrange("z j i -> z (j i)")

        def z_chunk_dma(c):
            zc = zpool.tile([128, CH, DZ], F16, tag="zc", name=f"zc{c}")
            # 2D AP: per-partition the (j, i) range is one contiguous run
            nc.sync.dma_start(
                out=zc.rearrange("p j i -> p (j i)"),
                in_=zT_flat[:, CH * c * SI:CH * (c + 1) * SI])
            return zc

        with (
            tc.tile_pool(name="wpool", bufs=2) as wpool,
            tc.tile_pool(name="apsum", bufs=2, space="PSUM") as apsum,
        ):
            w_sb = {}

            def w_load(nm, src):
                w_sb[nm] = wpool.tile([128, KT, DS], F16, tag="w", name=nm)
                nc.sync.dma_start(
                    out=w_sb[nm].rearrange("p m n -> p (m n)"), in_=src)

            w_load("wk", wkT16)
            w_load("wv", wvT16)

            zcs = [z_chunk_dma(c) for c in range(4)]

            # k/v shards for own rows: [128 i, 1024 d] each, then AllGather
            kv_sh = apool.tile([128, 2, DS], F16)
            for which, nm in ((0, "wk"), (1, "wv")):
                for n in range(2):
                    kvp = apsum.tile([128, 512], F32, tag="kvp")
                    for k in range(KT):
                        nc.tensor.matmul(kvp, sTi_sb[:, k, :],
                                         w_sb[nm][:, k, 512 * n:512 * (n + 1)],
                                         start=(k == 0), stop=(k == KT - 1))
                    nc.any.tensor_copy(kv_sh[:, which, 512 * n:512 * (n + 1)], kvp)

            w_load("wq", wqT16)
            w_load("wg", wgT16)

            nc.sync.dma_start(
                out=kv_agi.ap().rearrange("p (w n) -> p w n", w=2), in_=kv_sh)
            nc.gpsimd.collective_compute(
                "AllGather", AL.bypass, ins=[kv_agi.ap()], outs=[kv_ago.ap()],
                replica_groups=[list(range(NCORES))])

            # qT[d, i] += bq  (wq, bq pre-scaled by 1/8 on host);
            # bias is per-partition, so the add rides the ACT copy
            for m in range(KT):
                qp = apsum.tile([128, SI], F32, tag="qp")
                for k in range(KT):
                    nc.tensor.matmul(qp, w_sb["wq"][:, k, 128 * m:128 * (m + 1)],
                                     sTi_sb[:, k, :],
                                     start=(k == 0), stop=(k == KT - 1))
                nc.scalar.activation(qT_sb[:, m, :], qp, AF.Identity,
                                     bias=bq_sb[:, m:m + 1])

            # g = sigmoid(s_i @ wg^T)   [i, d]
            for n in range(2):
                gp = apsum.tile([128, 512], F32, tag="gp")
                for k in range(KT):
                    nc.tensor.matmul(gp, sTi_sb[:, k, :],
                                     w_sb["wg"][:, k, 512 * n:512 * (n + 1)],
                                     start=(k == 0), stop=(k == KT - 1))
                nc.scalar.activation(g16[:, 512 * n:512 * (n + 1)], gp,
                                     AF.Sigmoid)

        # ---------------- Phase B: z pipeline ----------------
        with (
            tc.tile_pool(name="sqpool", bufs=2) as sqpool,
            tc.tile_pool(name="ppsum", bufs=6, space="PSUM") as ppsum,
            tc.tile_pool(name="ktps", bufs=2, space="PSUM") as ktps,
            tc.tile_pool(name="stmp", bufs=2) as stmp,
        ):
            def finalize(c):
                # rs' = 1/sqrt(s1 - m1^2/128 + 128 eps); murs = m1 * rs'
                jsl = slice(CH * c, CH * (c + 1))
                m1 = stat[:, jsl, 0:1]
                s1 = stat[:, jsl, 1:2]
                rso = _bcast(rs, [list(rs.ap[0]), [1, CH], [0, 1]],
                             extra_offset=CH * c)
                mo = _bcast(murs, [list(murs.ap[0]), [1, CH], [0, 1]],
                            extra_offset=CH * c)
                t = stmp.tile([128, CH, 1], F32, tag="fin_t")
                nc.vector.tensor_tensor(out=t, in0=m1, in1=m1, op=AL.mult)
                v128 = stmp.tile([128, CH, 1], F32, tag="fin_v")
                nc.vector.scalar_tensor_tensor(
                    out=v128, in0=t, scalar=-1.0 / DZ, in1=s1,
                    op0=AL.mult, op1=AL.add)
                sq = stmp.tile([128, CH, 1], F32, tag="fin_s")
                nc.scalar.activation(sq, v128, AF.Sqrt, bias=eps_sb)
                nc.vector.reciprocal(rso, sq)
                nc.vector.tensor_tensor(out=mo, in0=m1, in1=rso, op=AL.mult)

            def correct(pb, j0):
                # bias16[:, :, j0:j0+BK] = rs*P - murs*c1   (all [128, H, BK])
                rs_rep = _bcast(rs, [list(rs.ap[0]), [0, H], [1, BK]],
                                extra_offset=j0)
                murs_rep = _bcast(murs, [list(murs.ap[0]), [0, H], [1, BK]],
                                  extra_offset=j0)
                pbv = _bcast(pb, [list(pb.ap[0]), [1, H], [NW, BK]])
                t1 = stmp.tile([128, H, BK], F16, tag="t1")
                nc.vector.tensor_tensor(out=t1, in0=pbv, in1=rs_rep, op=AL.mult)
                t2 = stmp.tile([128, H, BK], F16, tag="t2")
                nc.gpsimd.tensor_tensor(out=t2, in0=murs_rep, in1=c1m,
                                        op=AL.mult)
                nc.vector.tensor_tensor(out=bias16[:, :, j0:j0 + BK],
                                        in0=t1, in1=t2, op=AL.subtract)

            for c in range(NCH):
                j0c = CH * c
                zc = zcs[c] if c < 4 else z_chunk_dma(c)

                banks = []
                for s2 in range(CH // SQ):
                    zq = sqpool.tile([128, SQ, DZ], F16, tag="zq")
                    zsl = zc[:, SQ * s2:SQ * (s2 + 1), :]
                    # early chunks all-Vector (Scalar's queue head is still on
                    # phase A); then 8/28 Vector, rest Scalar
                    if c < 2 or (2 * c + s2) % 8 in (0, 4):
                        nc.vector.tensor_tensor(out=zq, in0=zsl, in1=zsl,
                                                op=AL.mult)
                    else:
                        nc.scalar.activation(zq, zsl, AF.Square)
                    for b in range(SQ // BK):
                        pb = ppsum.tile([128, BK, NW], F32, tag="pb")
                        jl0 = SQ * s2 + BK * b
                        for jj in range(BK):
                            nc.tensor.matmul(pb[:, jj, 0:NW - 1],
                                             zc[:, jl0 + jj, :], wza_sb,
                                             start=True, stop=True)
                            nc.tensor.matmul(pb[:, jj, NW - 1:NW],
                                             zq[:, jl0 - SQ * s2 + jj, :],
                                             ones_sb, start=True, stop=True)
                        nc.scalar.copy(
                            stat[:, j0c + jl0:j0c + jl0 + BK, :],
                            _bcast(pb, [list(pb.ap[0]), [NW, BK], [1, 2]],
                                   extra_offset=NW - 2))
                        banks.append((pb, j0c + jl0))
                finalize(c)
                for pb, j0 in banks:
                    correct(pb, j0)

                if c == 14:
                    # unpack the gathered k/v; build kT via PE transposes.
                    # Late enough that the AllGather (incl. inter-core skew)
                    # is done — a waiting DMA issue here would block every
                    # later z-chunk issue on the same queue.
                    kv_view = kv_ago.ap().rearrange(
                        "(t p) (w n) -> p t w n", p=128, w=2)
                    nc.sync.dma_start(out=v_sb, in_=kv_view[:, :, 1, :])
                    for m in range(KT):
                        knm = stmp.tile([128, KT, 128], F16, tag="knm")
                        nc.sync.dma_start(
                            out=knm, in_=kv_view[:, :, 0, 128 * m:128 * (m + 1)])
                        ktp = ktps.tile([128, KT, 128], F16, tag="ktp")
                        for t in range(KT):
                            nc.tensor.transpose(ktp[:, t, :], knm[:, t, :],
                                                ident16)
                        nc.any.tensor_copy(
                            kT_sb[:, m, :].rearrange("p (t n) -> p t n", n=128),
                            ktp)

        # ---------------- Phase C: attention ----------------
        with (
            tc.tile_pool(name="scps", bufs=2, space="PSUM") as scps,
            tc.tile_pool(name="atps", bufs=2, space="PSUM") as atps,
            tc.tile_pool(name="ops", bufs=1, space="PSUM") as ops,
            tc.tile_pool(name="attn", bufs=2) as attnp,
        ):
            ob = ops.tile([128, 2, 8, HD], F32)
            for h in range(H):
                m, p0 = h // 2, 64 * (h % 2)
                scp = scps.tile([128, 2, 512], F32, tag="scp")
                for n in range(2):
                    nc.tensor.matmul(scp[:, n, :],
                                     qT_sb[p0:p0 + 64, m, :],
                                     kT_sb[p0:p0 + 64, m, 512 * n:512 * (n + 1)],
                                     start=True, stop=True)
                sc_sb = attnp.tile([128, S], F32, tag="sc")
                nc.vector.tensor_tensor(
                    out=sc_sb, in0=scp.rearrange("p a b -> p (a b)"),
                    in1=bias16[:, h, :], op=AL.add)
                attn16 = attnp.tile([128, S], F16, tag="at")
                nc.scalar.activation(attn16, sc_sb, AF.Exp,
                                     accum_out=sums[:, h:h + 1])
                atb = atps.tile([128, 8, 128], F16, tag="atb")
                for t in range(8):
                    nc.tensor.transpose(atb[:, t, :],
                                        attn16[:, 128 * t:128 * (t + 1)],
                                        ident16)
                attnT = attnp.tile([128, 8, 128], F16, tag="atT")
                nc.any.tensor_copy(attnT, atb)
                for t in range(8):
                    nc.tensor.matmul(ob[:, h // 8, h % 8, :], attnT[:, t, :],
                                     v_sb[:, t, HD * h:HD * (h + 1)],
                                     start=(t == 0), stop=(t == 7))
                if h % 8 == 7:
                    hb = h // 8
                    nc.vector.reciprocal(inv[:, 8 * hb:8 * (hb + 1)],
                                         sums[:, 8 * hb:8 * (hb + 1)])
                    for hh in range(8 * hb, 8 * (hb + 1)):
                        nc.vector.scalar_tensor_tensor(
                            out=og16[:, HD * hh:HD * (hh + 1)],
                            in0=ob[:, hb, hh % 8, :],
                            scalar=inv[:, hh:hh + 1],
                            in1=g16[:, HD * hh:HD * (hh + 1)],
                            op0=AL.mult, op1=AL.mult)

        # ---------------- Phase D: output projection ----------------
        with (
            tc.tile_pool(name="wopool", bufs=1) as wopool,
            tc.tile_pool(name="dpsum", bufs=2, space="PSUM") as dpsum,
        ):
            wo_sb = wopool.tile([128, KT, DS], F16)
            nc.sync.dma_start(
                out=wo_sb.rearrange("p m n -> p (m n)"), in_=woT16)
            ogb = dpsum.tile([128, 8, 128], F16, tag="ogb")
            for t in range(8):
                nc.tensor.transpose(ogb[:, t, :],
                                    og16[:, 128 * t:128 * (t + 1)], ident16)
            nc.any.tensor_copy(ogT_sb.rearrange("p k n -> p (k n)"),
                               ogb.rearrange("p k n -> p (k n)"))
            for n in range(2):
                op_ = dpsum.tile([128, 512], F32, tag="op")
                for k in range(KT):
                    nc.tensor.matmul(op_, ogT_sb[:, k, :],
                                     wo_sb[:, k, 512 * n:512 * (n + 1)],
                                     start=(k == 0), stop=(k == KT - 1))
                nc.any.tensor_copy(out_sb[:, 512 * n:512 * (n + 1)], op_)
            nc.sync.dma_start(out=out_sh, in_=out_sb)


def prep_inputs(s, z, wq, bq, wk, wv, wg, z_norm_w, z_norm_b, wz, wo):
    """Host-side prep: shard + transpose/cast weights and z. Returns in_maps."""
    def _pm(wT):
        # [DS, N] -> [128, KT0*N]: partition p holds rows p, 128+p, ... packed
        # contiguously so the DMA is one 128-descriptor transfer
        n = wT.shape[1]
        return np.ascontiguousarray(
            wT.reshape(KT0, 128, n).transpose(1, 0, 2).reshape(128, KT0 * n))

    s2 = np.asarray(s)[0]                     # [S, DS]
    sT = np.ascontiguousarray(s2.T).astype(np.float16)
    wqT = _pm(np.ascontiguousarray((np.asarray(wq) / 8.0).T).astype(np.float16))
    wkT = _pm(np.ascontiguousarray(np.asarray(wk).T).astype(np.float16))
    wvT = _pm(np.ascontiguousarray(np.asarray(wv).T).astype(np.float16))
    wgT = _pm(np.ascontiguousarray(np.asarray(wg).T).astype(np.float16))
    woT = _pm(np.ascontiguousarray(np.asarray(wo).T).astype(np.float16))
    # wz folded with z_norm_w, scaled by sqrt(DZ); ones column appended.
    wz_f = (np.asarray(z_norm_w)[:, None] * np.asarray(wz).T) * np.sqrt(DZ)
    wz16 = wz_f.astype(np.float16)            # [DZ, H]
    wza = np.concatenate(
        [wz16, np.ones((DZ, 1), np.float16)], axis=1)  # [DZ, H+1]
    # c1x_h = sum_z wz_dev[z, h] / DZ  (f16-quantized wz to match device)
    c1x = (wz16.astype(np.float32).sum(axis=0) / DZ)[None, :].astype(np.float32)
    bq8 = (np.asarray(bq) / 8.0).astype(np.float32)[:, None]

    # z pre-transposed to [z, j, i] per core, f16.
    z0 = np.asarray(z)[0]                     # [S i, S j, DZ z]
    z16 = z0.astype(np.float16)
    zT = z16.transpose(2, 1, 0)               # [DZ z, S j, S i] (view)

    in_maps = []
    for c in range(NCORES):
        i0 = SI * c
        in_maps.append({
            "zT_sh": np.ascontiguousarray(zT[:, :, i0:i0 + SI]),
            "sTi16": _pm(np.ascontiguousarray(sT[:, i0:i0 + SI])),
            "wqT16": wqT, "wkT16": wkT, "wvT16": wvT, "wgT16": wgT,
            "woT16": woT, "wza16": wza, "c1x": c1x, "bq8": bq8,
        })
    return in_maps


_NC_CACHE = None


def _get_nc():
    global _NC_CACHE
    if _NC_CACHE is None:
        _NC_CACHE = build_nc()
    return _NC_CACHE


def kernel(**inputs):
    from concourse.bass_utils import run_bass_kernel_spmd
    nc = _get_nc()
    in_maps = prep_inputs(**inputs)
    res = run_bass_kernel_spmd(nc, in_maps, core_ids=list(range(NCORES)))
    out = np.empty((1, S, DS), dtype=np.float32)
    for c in range(NCORES):
        out[0, SI * c:SI * (c + 1), :] = res.results[c]["out_sh"]
    return out
